# revision 1
# baseline (speedup 1.0000x reference)
"""MoE kernel for Trainium2 (8 NeuronCores), expert-parallel.

Strategy:
  - Host computes the (tiny) router: logits = x @ router_w in f64, softmax,
    top-2 expert indices + gate probs per token (verified to match
    jax.lax.top_k selection exactly on f32 ties-by-lower-index).
  - Tokens are gathered per routed expert on host (all-to-all dispatch done
    at input-sharding time). Core e receives its expert's tokens padded to
    capacity C (max expert load rounded to 128).
  - The shared expert is split along the FFN dim F: core e owns columns
    [e*512,(e+1)*512) of S_up and the matching rows of S_down, and computes
    a partial shared output for ALL tokens; the host sums the 8 partials
    (a sum over F-slices is exact in the FFN structure since only gelu is
    nonlinear and it is applied per-F-element before the down projection).
  - Device kernel per core, two phases with all weights SBUF-resident:
      phase S: partial shared FFN over all 8192 tokens (F-slice 512)
      phase R: own routed expert over C gathered tokens, gate fused into
               the PSUM eviction
    Matmuls in bf16 with f32 PSUM accumulation; exact-erf gelu on ScalarE.
    Phase S weights are tiny (4MB) so compute starts almost immediately;
    the 16MB routed weights stream in on the SWDGE queue behind it.
  - Host combines: y = x + sum_cores shared_partial + gather of gated
    routed outputs (each token's top-2 expert rows).
"""

import sys

if "/opt/trn_rl_repo" not in sys.path:
    sys.path.insert(0, "/opt/trn_rl_repo")

from contextlib import ExitStack

import ml_dtypes
import numpy as np

H, F, E, TOPK = 1024, 4096, 8, 2
N_CORES = 8
CHUNK = 256  # tokens per pipeline chunk (2 c-tiles of 128)
NOUT = 2  # h-output tiles of 512
FS = F // N_CORES  # shared-expert F-slice per core (512)
BF16 = ml_dtypes.bfloat16

_nc_cache = {}

# test-harness hooks (unused when graded): set TRACE=True to request an NTFF
# profile; the BassKernelResults of the last run lands in LAST_RESULT.
TRACE = False
LAST_RESULT = None


def _ffn_phase(nc, tile, dt, act, *, wu, wd, x_r, out_r, c_lo, c_hi, n_f,
               pools, g_sb=None, g_base=0, paced_dmas=None):
    """One dense FFN phase: out = [gate *] gelu(x @ Wup) @ Wdown.

    wu: list of k-tiles [128, n_f*128] (lhsT slices along H)
    wd: list of n_f tiles [128, H]
    x_r/out_r: DRAM APs [128, kt, tokens] / [128, tokens/128, H]
    """
    import concourse.mybir as mybir

    xpool, hpool, opool, pup, pdown = pools
    KT_H = H // 128
    GELU = getattr(mybir.ActivationFunctionType, act)

    n_chunks = -(-(c_hi - c_lo) // CHUNK)
    for ic, c0 in enumerate(range(c_lo, c_hi, CHUNK)):
        cc = min(CHUNK, c_hi - c0)
        nct = cc // 128
        x_sb = xpool.tile([128, KT_H, CHUNK], dt.bfloat16, tag="x")
        x_dma = nc.sync.dma_start(x_sb[:, :, :cc], x_r[:, :, c0 : c0 + cc])
        # one single-bank PSUM tile per (ci,ho) output slice: gives each
        # slice its own semaphore, so evictions start as soon as that
        # slice's accumulation stops and the next chunk's first down
        # matmuls wait only on their own slice's eviction.
        ps_d = [
            pdown.tile([128, 512], dt.float32, tag=f"pd{s}", name=f"pd{s}")
            for s in range(nct * NOUT)
        ]
        if paced_dmas:
            # pace bulk background DMAs (next phase's weights) across this
            # phase: emit a slice per chunk, gated on this chunk's x arrival
            # so they don't hog HBM bandwidth ahead of the compute stream.
            from concourse.bass import _add_dep_helper

            # skip the first chunks entirely: they prime the compute pipeline
            # and any HBM contention there stalls the PE directly
            skip = min(2, n_chunks - 1)
            span = n_chunks - skip
            lo = len(paced_dmas) * max(0, ic - skip) // span
            hi = len(paced_dmas) * max(0, ic - skip + 1) // span
            for fn in paced_dmas[lo:hi]:
                w_dma = fn()
                _add_dep_helper(
                    w_dma.ins, x_dma.ins, True, "paced background weight DMA"
                )
        # f-loop pipelined by one step: down(f) is emitted after up(f+1) so
        # the gelu -> LDWEIGHTS(hT) chain of step f hides under the up
        # matmuls of step f+1 instead of stalling the first down matmul.
        def emit_up(f):
            ps_u = pup.tile([128, cc], dt.float32, tag="pu")
            for kt in range(KT_H):
                nc.tensor.matmul(
                    ps_u[:],
                    wu[kt][:, f * 128 : (f + 1) * 128],
                    x_sb[:, kt, :cc],
                    start=(kt == 0),
                    stop=(kt == KT_H - 1),
                )
            hT = hpool.tile([128, cc], dt.bfloat16, tag="h")
            nc.scalar.activation(hT[:], ps_u[:], GELU)
            return hT

        def emit_down(f, hT):
            for ci in range(nct):
                for ho in range(NOUT):
                    nc.tensor.matmul(
                        ps_d[ci * NOUT + ho][:],
                        hT[:, ci * 128 : (ci + 1) * 128],
                        wd[f][:, ho * 512 : (ho + 1) * 512],
                        start=(f == 0),
                        stop=(f == n_f - 1),
                    )

        depth = 2 if n_f > 2 else 1
        hts = [emit_up(f) for f in range(min(depth, n_f))]
        for f in range(depth, n_f):
            hts.append(emit_up(f))
            emit_down(f - depth, hts[f - depth])
        for f in range(max(0, n_f - depth), n_f):
            emit_down(f, hts[f])

        for ci in range(nct):
            n = (c0 - c_lo) // 128 + ci
            o_sb = opool.tile([128, H], dt.float32, tag="o")
            for ho in range(NOUT):
                dst = o_sb[:, ho * 512 : (ho + 1) * 512]
                src = ps_d[ci * NOUT + ho][:]
                # split evictions across DVE and ACT (Copy/Identity share the
                # gelu PWP table set, so no table reload) — halves the
                # eviction latency the next chunk's down matmuls wait on
                if g_sb is not None:
                    g = g_sb[:, g_base + n : g_base + n + 1]
                    if ho % 2 == 0:
                        nc.vector.tensor_scalar_mul(dst, src, g)
                    else:
                        nc.scalar.activation(
                            dst, src, mybir.ActivationFunctionType.Copy, scale=g
                        )
                else:
                    if ho % 2 == 0:
                        nc.vector.tensor_copy(dst, src)
                    else:
                        nc.scalar.activation(
                            dst, src, mybir.ActivationFunctionType.Copy
                        )
            nc.sync.dma_start(out_r[:, n, :], o_sb[:])


def _build_nc(c_routed, t_total, act="Gelu"):
    import concourse.mybir as mybir
    import concourse.tile as tile
    from concourse import bacc

    dt = mybir.dt
    assert c_routed % 128 == 0 and t_total % CHUNK == 0
    KT_H = H // 128  # 8 k-tiles along H
    KT_F = F // 128  # 32 k-tiles along F (routed down-proj)
    NF_S = FS // 128  # 4 f-tiles in the shared slice

    # Bacc (not raw Bass): its compile pass splits sync waits down to the
    # TRN2 limit of 1 wait per instruction (walrus rejects multi-wait IR).
    nc = bacc.Bacc(None, target_bir_lowering=False)
    xT_r = nc.dram_tensor("xT_r", [H, c_routed], dt.bfloat16, kind="ExternalInput")
    xT_s = nc.dram_tensor("xT_s", [H, t_total], dt.bfloat16, kind="ExternalInput")
    gates = nc.dram_tensor(
        "gates", [128, c_routed // 128], dt.float32, kind="ExternalInput"
    )
    w_up = nc.dram_tensor("w_up", [H, F], dt.bfloat16, kind="ExternalInput")
    w_down = nc.dram_tensor("w_down", [F, H], dt.bfloat16, kind="ExternalInput")
    su_s = nc.dram_tensor("su_s", [H, FS], dt.bfloat16, kind="ExternalInput")
    sd_s = nc.dram_tensor("sd_s", [FS, H], dt.bfloat16, kind="ExternalInput")
    out_r = nc.dram_tensor("out_r", [c_routed, H], dt.float32, kind="ExternalOutput")
    out_s = nc.dram_tensor("out_s", [t_total, H], dt.float32, kind="ExternalOutput")

    xTr_t = xT_r.rearrange("(kt p) c -> p kt c", p=128)
    xTs_t = xT_s.rearrange("(kt p) c -> p kt c", p=128)
    outr_t = out_r.rearrange("(n p) h -> p n h", p=128)
    outs_t = out_s.rearrange("(n p) h -> p n h", p=128)

    with tile.TileContext(nc) as tc, ExitStack() as ctx:
        swpool = ctx.enter_context(tc.tile_pool(name="sweights", bufs=1))
        wpool = ctx.enter_context(tc.tile_pool(name="weights", bufs=1))
        xpool = ctx.enter_context(tc.tile_pool(name="x", bufs=3))
        hpool = ctx.enter_context(tc.tile_pool(name="h", bufs=6))
        cpool = ctx.enter_context(tc.tile_pool(name="const", bufs=1))
        opool = ctx.enter_context(tc.tile_pool(name="out", bufs=3))
        # 4 psd slices + 3 pup bufs = 7 of 8 PSUM banks; bufs=4 (all 8 banks)
        # crashes the device (NRT_EXEC_UNIT_UNRECOVERABLE) — do not fill PSUM.
        pup = ctx.enter_context(tc.tile_pool(name="pup", bufs=3, space="PSUM"))
        pdown = ctx.enter_context(tc.tile_pool(name="pdown", bufs=1, space="PSUM"))
        pools = (xpool, hpool, opool, pup, pdown)

        # shared-slice weights (small, on the HWDGE queue -> available fast);
        # one coalesced DMA each so SP-sequencer dispatch doesn't delay the
        # first x-chunk load behind a dozen small descriptors
        su_all = swpool.tile([128, KT_H, FS], dt.bfloat16, tag="su")
        nc.sync.dma_start(su_all[:], su_s.rearrange("(kt p) f -> p kt f", p=128)[:])
        su = [su_all[:, kt, :] for kt in range(KT_H)]
        sd_all = swpool.tile([128, NF_S, H], dt.bfloat16, tag="sd")
        nc.sync.dma_start(sd_all[:], sd_s.rearrange("(ft p) h -> p ft h", p=128)[:])
        sd = [sd_all[:, ft, :] for ft in range(NF_S)]

        # routed weights (16MB): tiles allocated now, DMAs deferred — they
        # are emitted paced across the shared phase (on the SWDGE queue) so
        # they don't steal HBM bandwidth from the shared phase's startup.
        wu, wd, w_dma_fns = [], [], []
        wu_t = w_up.rearrange("(kt p) f -> p kt f", p=128)
        for kt in range(KT_H):
            t = wpool.tile([128, F], dt.bfloat16, tag=f"wu{kt}")
            w_dma_fns.append(
                lambda t=t, kt=kt: nc.gpsimd.dma_start(t[:], wu_t[:, kt, :])
            )
            wu.append(t)
        wd_t = w_down.rearrange("(ft p) h -> p ft h", p=128)
        for ft in range(KT_F):
            t = wpool.tile([128, H], dt.bfloat16, tag=f"wd{ft}")
            w_dma_fns.append(
                lambda t=t, ft=ft: nc.gpsimd.dma_start(t[:], wd_t[:, ft, :])
            )
            wd.append(t)

        # phase S: partial shared FFN over all tokens, F-slice FS
        _ffn_phase(nc, tile, dt, act, wu=su, wd=sd, x_r=xTs_t, out_r=outs_t,
                   c_lo=0, c_hi=t_total, n_f=NF_S, pools=pools,
                   paced_dmas=w_dma_fns)

        g_sb = cpool.tile([128, c_routed // 128], dt.float32)
        nc.sync.dma_start(g_sb[:], gates[:])
        # phase R: routed expert over gathered tokens, gated eviction
        _ffn_phase(nc, tile, dt, act, wu=wu, wd=wd, x_r=xTr_t, out_r=outr_t,
                   c_lo=0, c_hi=c_routed, n_f=KT_F, pools=pools,
                   g_sb=g_sb)

    nc.finalize()
    return nc


def _get_nc(c_routed, t_total):
    key = (c_routed, t_total)
    if key not in _nc_cache:
        _nc_cache[key] = _build_nc(c_routed, t_total)
    return _nc_cache[key]


def _route(xf, router_w):
    """Host router in f64: top-2 indices (jax tie-break: lower index first)
    and their softmax probs."""
    logits = xf.astype(np.float64) @ router_w.astype(np.float64)
    m = logits.max(-1, keepdims=True)
    p = np.exp(logits - m)
    p /= p.sum(-1, keepdims=True)
    order = np.argsort(-p, axis=-1, kind="stable")
    top_idx = order[:, :TOPK]
    top_p = np.take_along_axis(p, top_idx, -1).astype(np.float32)
    return top_idx, top_p


def kernel(**inputs):
    x = np.ascontiguousarray(np.asarray(inputs["x"], np.float32))
    shared_up = np.asarray(inputs["shared_up"], np.float32)[0]
    shared_down = np.asarray(inputs["shared_down"], np.float32)[0]
    routed_up = np.asarray(inputs["routed_up"], np.float32)
    routed_down = np.asarray(inputs["routed_down"], np.float32)
    router_w = np.asarray(inputs["router_w"], np.float32)

    B, S, _ = x.shape
    T = B * S
    xf = x.reshape(T, H)

    top_idx, top_p = _route(xf, router_w)

    token_lists = [np.where((top_idx == e).any(-1))[0] for e in range(E)]
    c_cap = max(128, -(-max(len(l) for l in token_lists) // 128) * 128)

    # position of (token, slot) inside its expert's gathered buffer
    pos = np.zeros((T, TOPK), np.int64)
    gates_per_e = np.zeros((E, c_cap), np.float32)
    for e in range(E):
        lst = token_lists[e]
        for k in range(TOPK):
            sel = np.where(top_idx[:, k] == e)[0]
            p_in = np.searchsorted(lst, sel)
            pos[sel, k] = p_in
            gates_per_e[e, p_in] = top_p[sel, k]

    xf_bf = xf.astype(BF16)
    xTs = np.ascontiguousarray(xf_bf.T)  # [H, T], shared phase input
    su_bf = shared_up.astype(BF16)
    sd_bf = shared_down.astype(BF16)

    in_maps = []
    for e in range(E):
        lst = token_lists[e]
        xe = np.zeros((c_cap, H), BF16)
        xe[: len(lst)] = xf_bf[lst]
        in_maps.append(
            {
                "xT_r": np.ascontiguousarray(xe.T),
                "xT_s": xTs,
                "gates": np.ascontiguousarray(
                    gates_per_e[e].reshape(c_cap // 128, 128).T
                ),
                "w_up": routed_up[e].astype(BF16),
                "w_down": routed_down[e].astype(BF16),
                "su_s": np.ascontiguousarray(su_bf[:, e * FS : (e + 1) * FS]),
                "sd_s": np.ascontiguousarray(sd_bf[e * FS : (e + 1) * FS, :]),
            }
        )

    from concourse.bass_utils import run_bass_kernel_spmd

    nc = _get_nc(c_cap, T)
    res = run_bass_kernel_spmd(nc, in_maps, list(range(N_CORES)), trace=TRACE)
    global LAST_RESULT
    LAST_RESULT = res

    y = xf.copy()
    for e in range(E):
        y += res.results[e]["out_s"]
    y_routed = np.stack([res.results[e]["out_r"] for e in range(E)])  # gated rows
    for k in range(TOPK):
        y += y_routed[top_idx[:, k], pos[:, k]]
    return y.reshape(B, S, H)



# revision 6
# speedup vs baseline: 1.5085x; 1.5085x over previous
"""MoE kernel for Trainium2 (8 NeuronCores), expert-parallel.

Strategy:
  - Host computes the (tiny) router: logits = x @ router_w in f64, softmax,
    top-2 expert indices + gate probs per token (verified to match
    jax.lax.top_k selection exactly on f32 ties-by-lower-index).
  - Tokens are gathered per routed expert on host (all-to-all dispatch done
    at input-sharding time). Core e receives its expert's tokens padded to
    capacity C (max expert load rounded to 128).
  - The shared expert is split along the FFN dim F: core e owns columns
    [e*512,(e+1)*512) of S_up and the matching rows of S_down, and computes
    a partial shared output for ALL tokens; the host sums the 8 partials
    (exact in the FFN structure since gelu is applied per-F-element before
    the down projection). Shared phase runs in bf16: the shared output is
    the largest non-residual term, and a single fp8 quantization point
    anywhere in its path costs ~1.2e-2 of final rel err (measured) — too
    close to the 2e-2 gate.
  - The routed phase runs in fp8 e4m3 with perf_mode=DoubleRow (2 fp8
    weights per PE cell -> ~1.45x bf16 matmul throughput). Scales: x*16,
    W*32, folded back via the gelu activation scale (1/512) and the gates
    (1/32). Measured end-to-end rel err contribution: ~8e-3 (gated routed
    outputs are ~3x smaller than the shared output, so fp8 is safe here).
  - Device kernel per core, two phases with all weights SBUF-resident:
      phase S: partial shared FFN over all 8192 tokens (F-slice 512), bf16
      phase R: own routed expert over C gathered tokens, fp8 DoubleRow,
               gate fused into the PSUM eviction
    Chunks of 512 tokens; the f-loop computes all gelu tiles for a chunk
    (h resident in SBUF), then the down matmuls loop f-inner per 128-token
    group so each PSUM bank accumulates the full F contraction (keeps PSUM
    use at 7 banks: 3 pup + 4 pdown; 8 banks crashes the device).
    Chunk-level software pipelining: up(c+1) is emitted before down(c) so
    the last-gelu -> first-down-matmul dependency of chunk c hides under
    the up matmuls of chunk c+1.
    Phase S weights are tiny (2MB) so compute starts almost immediately;
    the 8MB fp8 routed weights stream in on the SWDGE queue behind it.
  - All outputs are written in bf16 (halves writeback traffic; adds ~2e-3
    partial-sum rounding, measured harmless). Host combines in f32:
    y = x + sum_cores shared_partial + gather of gated routed rows.
"""

import sys

if "/opt/trn_rl_repo" not in sys.path:
    sys.path.insert(0, "/opt/trn_rl_repo")

from contextlib import ExitStack

import ml_dtypes
import numpy as np

H, F, E, TOPK = 1024, 4096, 8, 2
N_CORES = 8
CHUNK = 512  # tokens per pipeline chunk (moving-operand FD / one PSUM bank)
NOUT = 2  # h-output tiles of 512
FS = F // N_CORES  # shared-expert F-slice per core (512)
BF16 = ml_dtypes.bfloat16
FP8 = ml_dtypes.float8_e4m3fn
SX = 16.0  # fp8 scale on x
SW = 32.0  # fp8 scale on routed weights

_nc_cache = {}

# test-harness hooks (unused when graded): set TRACE=True to request an NTFF
# profile; the BassKernelResults of the last run lands in LAST_RESULT.
TRACE = False
LAST_RESULT = None


def _ffn_phase(nc, tile, dt, *, wu, wd, x_r, out_r, c_lo, c_hi, n_f, kstep,
               pools, htag, hdt, xdt, gelu_scale=1.0, g_sb=None, g_base=0,
               paced_dmas=None):
    """One dense FFN phase: out = [gate *] gelu(scale * (x @ Wup)) @ Wdown.

    wu: SBUF tile [128, KT_H, n_f*128] (lhsT, k-tiles along H)
    wd: SBUF tile [128, n_f, H]
    x_r/out_r: DRAM APs [128, kt, tokens] / [128, tokens/128, H]
    kstep: 1 for bf16, 2 for fp8 DoubleRow (pairs of k-planes per matmul)
    """
    import concourse.mybir as mybir

    xpool, hpool, opool, pup, pdown = pools
    KT_H = H // 128
    GELU = mybir.ActivationFunctionType.Gelu
    perf_mode = mybir.MatmulPerfMode.DoubleRow if kstep == 2 else None
    n_hp = n_f // kstep  # h pair-tiles per chunk

    def emit_up_chunk(ic, c0, cc):
        x_sb = xpool.tile([128, KT_H, CHUNK], xdt, tag=f"x{htag}")
        x_dma = nc.sync.dma_start(x_sb[:, :, :cc], x_r[:, :, c0 : c0 + cc])
        if paced_dmas:
            # pace bulk background DMAs (next phase's weights) across this
            # phase: emit a slice per chunk, gated on this chunk's x arrival
            # so they don't hog HBM bandwidth ahead of the compute stream.
            from concourse.bass import _add_dep_helper

            n_chunks = -(-(c_hi - c_lo) // CHUNK)
            skip = min(2, n_chunks - 1)
            span = n_chunks - skip
            lo = len(paced_dmas) * max(0, ic - skip) // span
            hi = len(paced_dmas) * max(0, ic - skip + 1) // span
            for fn in paced_dmas[lo:hi]:
                w_dma = fn()
                _add_dep_helper(
                    w_dma.ins, x_dma.ins, True, "paced background weight DMA"
                )
        hts = []
        for fp in range(n_hp):
            ht = hpool.tile([128, kstep, CHUNK], hdt, tag=f"{htag}{fp}")
            hts.append(ht)
        for f in range(n_f):
            ps_u = pup.tile([128, CHUNK], dt.float32, tag="pu")
            for kt in range(0, KT_H, kstep):
                if kstep == 2:
                    lhsT = wu[:, kt : kt + 2, f * 128 : (f + 1) * 128]
                    rhs = x_sb[:, kt : kt + 2, :cc]
                else:
                    lhsT = wu[:, kt, f * 128 : (f + 1) * 128]
                    rhs = x_sb[:, kt, :cc]
                nc.tensor.matmul(
                    ps_u[:, :cc], lhsT, rhs,
                    start=(kt == 0),
                    stop=(kt + kstep >= KT_H),
                    perf_mode=perf_mode,
                )
            nc.scalar.activation(
                hts[f // kstep][:, f % kstep, :cc], ps_u[:, :cc], GELU,
                scale=gelu_scale,
            )
        return hts

    def emit_down_chunk(c0, cc, hts):
        nct = cc // 128
        for ci in range(nct):
            n = (c0 - c_lo) // 128 + ci
            o_sb = opool.tile([128, H], dt.bfloat16, tag="o")
            for ho in range(NOUT):
                psd = pdown.tile([128, 512], dt.float32, tag=f"pd{ho}")
                for fp in range(n_hp):
                    if kstep == 2:
                        lhsT = hts[fp][:, :, ci * 128 : (ci + 1) * 128]
                        rhs = wd[:, fp * 2 : fp * 2 + 2, ho * 512 : (ho + 1) * 512]
                    else:
                        lhsT = hts[fp][:, 0, ci * 128 : (ci + 1) * 128]
                        rhs = wd[:, fp, ho * 512 : (ho + 1) * 512]
                    nc.tensor.matmul(
                        psd[:], lhsT, rhs,
                        start=(fp == 0),
                        stop=(fp == n_hp - 1),
                        perf_mode=perf_mode,
                    )
                dst = o_sb[:, ho * 512 : (ho + 1) * 512]
                # split evictions across DVE and ACT (Copy shares the gelu
                # PWP table set, so no table reload)
                if g_sb is not None:
                    g = g_sb[:, g_base + n : g_base + n + 1]
                    if ho % 2 == 0:
                        nc.vector.tensor_scalar_mul(dst, psd[:], g)
                    else:
                        nc.scalar.activation(
                            dst, psd[:], mybir.ActivationFunctionType.Copy,
                            scale=g,
                        )
                else:
                    if ho % 2 == 0:
                        nc.vector.tensor_copy(dst, psd[:])
                    else:
                        nc.scalar.activation(
                            dst, psd[:], mybir.ActivationFunctionType.Copy
                        )
            nc.sync.dma_start(out_r[:, n, :], o_sb[:])

    prev = None
    for ic, c0 in enumerate(range(c_lo, c_hi, CHUNK)):
        cc = min(CHUNK, c_hi - c0)
        hts = emit_up_chunk(ic, c0, cc)
        if prev is not None:
            emit_down_chunk(*prev)
        prev = (c0, cc, hts)
    emit_down_chunk(*prev)


def _build_nc(c_routed, t_total):
    import concourse.mybir as mybir
    import concourse.tile as tile
    from concourse import bacc

    dt = mybir.dt
    assert c_routed % 128 == 0 and t_total % CHUNK == 0
    KT_H = H // 128  # 8 k-tiles along H
    KT_F = F // 128  # 32 f-tiles (routed down-proj)
    NF_S = FS // 128  # 4 f-tiles in the shared slice

    # Bacc (not raw Bass): its compile pass splits sync waits down to the
    # TRN2 limit of 1 wait per instruction (walrus rejects multi-wait IR).
    nc = bacc.Bacc(None, target_bir_lowering=False)
    xT_r = nc.dram_tensor("xT_r", [H, c_routed], dt.float8e4, kind="ExternalInput")
    xT_s = nc.dram_tensor("xT_s", [H, t_total], dt.bfloat16, kind="ExternalInput")
    gates = nc.dram_tensor(
        "gates", [128, c_routed // 128], dt.float32, kind="ExternalInput"
    )
    w_up = nc.dram_tensor("w_up", [H, F], dt.float8e4, kind="ExternalInput")
    w_down = nc.dram_tensor("w_down", [F, H], dt.float8e4, kind="ExternalInput")
    su_s = nc.dram_tensor("su_s", [H, FS], dt.bfloat16, kind="ExternalInput")
    sd_s = nc.dram_tensor("sd_s", [FS, H], dt.bfloat16, kind="ExternalInput")
    out_r = nc.dram_tensor("out_r", [c_routed, H], dt.bfloat16, kind="ExternalOutput")
    out_s = nc.dram_tensor("out_s", [t_total, H], dt.bfloat16, kind="ExternalOutput")

    xTr_t = xT_r.rearrange("(kt p) c -> p kt c", p=128)
    xTs_t = xT_s.rearrange("(kt p) c -> p kt c", p=128)
    outr_t = out_r.rearrange("(n p) h -> p n h", p=128)
    outs_t = out_s.rearrange("(n p) h -> p n h", p=128)

    with tile.TileContext(nc) as tc, ExitStack() as ctx:
        swpool = ctx.enter_context(tc.tile_pool(name="sweights", bufs=1))
        wpool = ctx.enter_context(tc.tile_pool(name="weights", bufs=1))
        xpool = ctx.enter_context(tc.tile_pool(name="x", bufs=3))
        hpool = ctx.enter_context(tc.tile_pool(name="h", bufs=2))
        cpool = ctx.enter_context(tc.tile_pool(name="const", bufs=1))
        opool = ctx.enter_context(tc.tile_pool(name="out", bufs=6))
        # 3 pup + 4 pdown = 7 of 8 PSUM banks; using all 8 banks crashes the
        # device (NRT_EXEC_UNIT_UNRECOVERABLE) — do not fill PSUM.
        pup = ctx.enter_context(tc.tile_pool(name="pup", bufs=3, space="PSUM"))
        pdown = ctx.enter_context(tc.tile_pool(name="pdown", bufs=2, space="PSUM"))
        pools = (xpool, hpool, opool, pup, pdown)

        # shared-slice weights (small, on the HWDGE queue -> available fast);
        # one coalesced DMA each so SP-sequencer dispatch doesn't delay the
        # first x-chunk load behind a dozen small descriptors
        su_all = swpool.tile([128, KT_H, FS], dt.bfloat16, tag="su")
        nc.sync.dma_start(su_all[:], su_s.rearrange("(kt p) f -> p kt f", p=128)[:])
        sd_all = swpool.tile([128, NF_S, H], dt.bfloat16, tag="sd")
        nc.sync.dma_start(sd_all[:], sd_s.rearrange("(ft p) h -> p ft h", p=128)[:])

        # routed fp8 weights (8MB): tiles allocated now, DMAs deferred — they
        # are emitted paced across the shared phase (on the SWDGE queue) so
        # they don't steal HBM bandwidth from the shared phase's startup.
        w_dma_fns = []
        wu_all = wpool.tile([128, KT_H, F], dt.float8e4, tag="wu")
        wu_t = w_up.rearrange("(kt p) f -> p kt f", p=128)
        for kt in range(KT_H):
            w_dma_fns.append(
                lambda kt=kt: nc.gpsimd.dma_start(wu_all[:, kt, :], wu_t[:, kt, :])
            )
        wd_all = wpool.tile([128, KT_F, H], dt.float8e4, tag="wd")
        wd_t = w_down.rearrange("(ft p) h -> p ft h", p=128)
        for ft in range(0, KT_F, 4):
            w_dma_fns.append(
                lambda ft=ft: nc.gpsimd.dma_start(
                    wd_all[:, ft : ft + 4, :], wd_t[:, ft : ft + 4, :]
                )
            )

        # phase S: partial shared FFN over all tokens, F-slice FS, bf16
        _ffn_phase(nc, tile, dt, wu=su_all, wd=sd_all, x_r=xTs_t, out_r=outs_t,
                   c_lo=0, c_hi=t_total, n_f=NF_S, kstep=1, pools=pools,
                   htag="sh", hdt=dt.bfloat16, xdt=dt.bfloat16,
                   paced_dmas=w_dma_fns)

        g_sb = cpool.tile([128, c_routed // 128], dt.float32)
        nc.sync.dma_start(g_sb[:], gates[:])
        # phase R: routed expert over gathered tokens, fp8 DoubleRow, gated
        _ffn_phase(nc, tile, dt, wu=wu_all, wd=wd_all, x_r=xTr_t, out_r=outr_t,
                   c_lo=0, c_hi=c_routed, n_f=KT_F, kstep=2, pools=pools,
                   htag="rh", hdt=dt.float8e4, xdt=dt.float8e4,
                   gelu_scale=1.0 / (SX * SW), g_sb=g_sb)

    nc.finalize()
    return nc


def _get_nc(c_routed, t_total):
    key = (c_routed, t_total)
    if key not in _nc_cache:
        _nc_cache[key] = _build_nc(c_routed, t_total)
    return _nc_cache[key]


def _route(xf, router_w):
    """Host router in f64: top-2 indices (jax tie-break: lower index first)
    and their softmax probs."""
    logits = xf.astype(np.float64) @ router_w.astype(np.float64)
    m = logits.max(-1, keepdims=True)
    p = np.exp(logits - m)
    p /= p.sum(-1, keepdims=True)
    order = np.argsort(-p, axis=-1, kind="stable")
    top_idx = order[:, :TOPK]
    top_p = np.take_along_axis(p, top_idx, -1).astype(np.float32)
    return top_idx, top_p


def _fp8(v, scale):
    return np.clip(v * scale, -240.0, 240.0).astype(FP8)


def kernel(**inputs):
    x = np.ascontiguousarray(np.asarray(inputs["x"], np.float32))
    shared_up = np.asarray(inputs["shared_up"], np.float32)[0]
    shared_down = np.asarray(inputs["shared_down"], np.float32)[0]
    routed_up = np.asarray(inputs["routed_up"], np.float32)
    routed_down = np.asarray(inputs["routed_down"], np.float32)
    router_w = np.asarray(inputs["router_w"], np.float32)

    B, S, _ = x.shape
    T = B * S
    xf = x.reshape(T, H)

    top_idx, top_p = _route(xf, router_w)

    token_lists = [np.where((top_idx == e).any(-1))[0] for e in range(E)]
    c_cap = max(128, -(-max(len(l) for l in token_lists) // 128) * 128)

    # position of (token, slot) inside its expert's gathered buffer
    pos = np.zeros((T, TOPK), np.int64)
    gates_per_e = np.zeros((E, c_cap), np.float32)
    for e in range(E):
        lst = token_lists[e]
        for k in range(TOPK):
            sel = np.where(top_idx[:, k] == e)[0]
            p_in = np.searchsorted(lst, sel)
            pos[sel, k] = p_in
            gates_per_e[e, p_in] = top_p[sel, k]
    gates_per_e *= 1.0 / SW  # fold the fp8 down-weight scale into the gates

    xf_bf = xf.astype(BF16)
    xTs = np.ascontiguousarray(xf_bf.T)  # [H, T], shared phase input
    xf_q = _fp8(xf, SX)  # [T, H] fp8, routed phase input
    su_bf = shared_up.astype(BF16)
    sd_bf = shared_down.astype(BF16)

    in_maps = []
    for e in range(E):
        lst = token_lists[e]
        xe = np.zeros((c_cap, H), FP8)
        xe[: len(lst)] = xf_q[lst]
        in_maps.append(
            {
                "xT_r": np.ascontiguousarray(xe.T),
                "xT_s": xTs,
                "gates": np.ascontiguousarray(
                    gates_per_e[e].reshape(c_cap // 128, 128).T
                ),
                "w_up": _fp8(routed_up[e], SW),
                "w_down": _fp8(routed_down[e], SW),
                "su_s": np.ascontiguousarray(su_bf[:, e * FS : (e + 1) * FS]),
                "sd_s": np.ascontiguousarray(sd_bf[e * FS : (e + 1) * FS, :]),
            }
        )

    from concourse.bass_utils import run_bass_kernel_spmd

    nc = _get_nc(c_cap, T)
    res = run_bass_kernel_spmd(nc, in_maps, list(range(N_CORES)), trace=TRACE)
    global LAST_RESULT
    LAST_RESULT = res

    y = xf.copy()
    for e in range(E):
        y += res.results[e]["out_s"].astype(np.float32)
    y_routed = np.stack(
        [res.results[e]["out_r"].astype(np.float32) for e in range(E)]
    )  # gated rows
    for k in range(TOPK):
        y += y_routed[top_idx[:, k], pos[:, k]]
    return y.reshape(B, S, H)


# revision 9
# speedup vs baseline: 1.5230x; 1.0096x over previous
"""MoE kernel for Trainium2 (8 NeuronCores), expert-parallel.

Strategy:
  - Host computes the (tiny) router: logits = x @ router_w in f64, softmax,
    top-2 expert indices + gate probs per token (verified to match
    jax.lax.top_k selection exactly on f32 ties-by-lower-index).
  - Tokens are gathered per routed expert on host (all-to-all dispatch done
    at input-sharding time). Core e receives its expert's tokens padded to
    capacity C (max expert load rounded to 128).
  - The shared expert is split along the FFN dim F: core e owns columns
    [e*512,(e+1)*512) of S_up and the matching rows of S_down, and computes
    a partial shared output for ALL tokens; the host sums the 8 partials
    (exact in the FFN structure since gelu is applied per-F-element before
    the down projection). Shared phase runs in bf16: the shared output is
    the largest non-residual term, and a single fp8 quantization point
    anywhere in its path costs ~1.2e-2 of final rel err (measured) — too
    close to the 2e-2 gate.
  - The routed phase runs in fp8 e4m3 with perf_mode=DoubleRow (2 fp8
    weights per PE cell -> ~1.45x bf16 matmul throughput). Scales: x*16,
    W*32, folded back via the gelu activation scale (1/512) and the gates
    (1/32). Measured end-to-end rel err contribution: ~8e-3 (gated routed
    outputs are ~3x smaller than the shared output, so fp8 is safe here).
  - Device kernel per core, two phases with all weights SBUF-resident:
      phase S: partial shared FFN over all 8192 tokens (F-slice 512), bf16
      phase R: own routed expert over C gathered tokens, fp8 DoubleRow,
               gate fused into the PSUM eviction
    Chunks of 512 tokens; the f-loop computes all gelu tiles for a chunk
    (h resident in SBUF), then the down matmuls loop f-inner per 128-token
    group so each PSUM bank accumulates the full F contraction (keeps PSUM
    use at 7 banks: 3 pup + 4 pdown; 8 banks crashes the device).
    Chunk-level software pipelining: up(c+1) is emitted before down(c) so
    the last-gelu -> first-down-matmul dependency of chunk c hides under
    the up matmuls of chunk c+1.
    Phase S weights are tiny (2MB) so compute starts almost immediately;
    the 8MB fp8 routed weights stream in on the SWDGE queue behind it.
  - All outputs are written in bf16 (halves writeback traffic; adds ~2e-3
    partial-sum rounding, measured harmless). Host combines in f32:
    y = x + sum_cores shared_partial + gather of gated routed rows.
"""

import sys

if "/opt/trn_rl_repo" not in sys.path:
    sys.path.insert(0, "/opt/trn_rl_repo")

from contextlib import ExitStack

import ml_dtypes
import numpy as np

H, F, E, TOPK = 1024, 4096, 8, 2
N_CORES = 8
CHUNK = 512  # tokens per pipeline chunk (moving-operand FD / one PSUM bank)
NOUT = 2  # h-output tiles of 512
FS = F // N_CORES  # shared-expert F-slice per core (512)
BF16 = ml_dtypes.bfloat16
FP8 = ml_dtypes.float8_e4m3fn
SX = 16.0  # fp8 scale on x
SW = 32.0  # fp8 scale on routed weights

_nc_cache = {}

# test-harness hooks (unused when graded): set TRACE=True to request an NTFF
# profile; the BassKernelResults of the last run lands in LAST_RESULT.
TRACE = False
LAST_RESULT = None


def _ffn_phase(nc, tile, dt, *, wu, wd, x_r, out_r, c_lo, c_hi, n_f, kstep,
               pools, htag, hdt, xdt, gelu_scale=1.0, g_sb=None, g_base=0,
               paced_dmas=None):
    """One dense FFN phase: out = [gate *] gelu(scale * (x @ Wup)) @ Wdown.

    wu: SBUF tile [128, KT_H, n_f*128] (lhsT, k-tiles along H)
    wd: SBUF tile [128, n_f, H]
    x_r/out_r: DRAM APs [128, kt, tokens] / [128, tokens/128, H]
    kstep: 1 for bf16, 2 for fp8 DoubleRow (pairs of k-planes per matmul)
    """
    import concourse.mybir as mybir

    xpool, hpool, opool, pup, pdown = pools
    KT_H = H // 128
    GELU = mybir.ActivationFunctionType.Gelu
    perf_mode = mybir.MatmulPerfMode.DoubleRow if kstep == 2 else None
    n_hp = n_f // kstep  # h pair-tiles per chunk

    def emit_up_chunk(ic, c0, cc):
        x_sb = xpool.tile([128, KT_H, CHUNK], xdt, tag=f"x{htag}")
        if ic == 0 and paced_dmas is not None:
            # very first chunk of the kernel: split the x load in halves so
            # the f=0 matmul chain can start on the first half while the
            # second transfers (weights stream in parallel on the ACT ring)
            nc.sync.dma_start(
                x_sb[:, : KT_H // 2, :cc], x_r[:, : KT_H // 2, c0 : c0 + cc]
            )
            x_dma = nc.sync.dma_start(
                x_sb[:, KT_H // 2 :, :cc], x_r[:, KT_H // 2 :, c0 : c0 + cc]
            )
        else:
            x_dma = nc.sync.dma_start(x_sb[:, :, :cc], x_r[:, :, c0 : c0 + cc])
        if paced_dmas:
            # pace bulk background DMAs (next phase's weights) across this
            # phase: emit a slice per chunk, gated on this chunk's x arrival
            # so they don't hog HBM bandwidth ahead of the compute stream.
            from concourse.bass import _add_dep_helper

            n_chunks = -(-(c_hi - c_lo) // CHUNK)
            skip = min(2, n_chunks - 1)
            span = n_chunks - skip
            lo = len(paced_dmas) * max(0, ic - skip) // span
            hi = len(paced_dmas) * max(0, ic - skip + 1) // span
            for fn in paced_dmas[lo:hi]:
                w_dma = fn()
                _add_dep_helper(
                    w_dma.ins, x_dma.ins, True, "paced background weight DMA"
                )
        hts = []
        for fp in range(n_hp):
            ht = hpool.tile([128, kstep, CHUNK], hdt, tag=f"{htag}{fp}")
            hts.append(ht)
        for f in range(n_f):
            ps_u = pup.tile([128, CHUNK], dt.float32, tag="pu")
            for kt in range(0, KT_H, kstep):
                if kstep == 2:
                    lhsT = wu[:, kt : kt + 2, f * 128 : (f + 1) * 128]
                    rhs = x_sb[:, kt : kt + 2, :cc]
                else:
                    lhsT = wu[:, kt, f * 128 : (f + 1) * 128]
                    rhs = x_sb[:, kt, :cc]
                nc.tensor.matmul(
                    ps_u[:, :cc], lhsT, rhs,
                    start=(kt == 0),
                    stop=(kt + kstep >= KT_H),
                    perf_mode=perf_mode,
                )
            nc.scalar.activation(
                hts[f // kstep][:, f % kstep, :cc], ps_u[:, :cc], GELU,
                scale=gelu_scale,
            )
        return hts

    def emit_down_chunk(c0, cc, hts):
        nct = cc // 128
        for ci in range(nct):
            n = (c0 - c_lo) // 128 + ci
            o_sb = opool.tile([128, H], dt.bfloat16, tag="o")
            for ho in range(NOUT):
                psd = pdown.tile([128, 512], dt.float32, tag=f"pd{ho}")
                for fp in range(n_hp):
                    if kstep == 2:
                        lhsT = hts[fp][:, :, ci * 128 : (ci + 1) * 128]
                        rhs = wd[:, fp * 2 : fp * 2 + 2, ho * 512 : (ho + 1) * 512]
                    else:
                        lhsT = hts[fp][:, 0, ci * 128 : (ci + 1) * 128]
                        rhs = wd[:, fp, ho * 512 : (ho + 1) * 512]
                    nc.tensor.matmul(
                        psd[:], lhsT, rhs,
                        start=(fp == 0),
                        stop=(fp == n_hp - 1),
                        perf_mode=perf_mode,
                    )
                dst = o_sb[:, ho * 512 : (ho + 1) * 512]
                # split evictions across DVE and ACT (Copy shares the gelu
                # PWP table set, so no table reload)
                if g_sb is not None:
                    g = g_sb[:, g_base + n : g_base + n + 1]
                    if ho % 2 == 0:
                        nc.vector.tensor_scalar_mul(dst, psd[:], g)
                    else:
                        nc.scalar.activation(
                            dst, psd[:], mybir.ActivationFunctionType.Copy,
                            scale=g,
                        )
                else:
                    if ho % 2 == 0:
                        nc.vector.tensor_copy(dst, psd[:])
                    else:
                        nc.scalar.activation(
                            dst, psd[:], mybir.ActivationFunctionType.Copy
                        )
            nc.sync.dma_start(out_r[:, n, :], o_sb[:])

    prev = None
    for ic, c0 in enumerate(range(c_lo, c_hi, CHUNK)):
        cc = min(CHUNK, c_hi - c0)
        hts = emit_up_chunk(ic, c0, cc)
        if prev is not None:
            emit_down_chunk(*prev)
        prev = (c0, cc, hts)
    emit_down_chunk(*prev)


def _build_nc(c_routed, t_total):
    import concourse.mybir as mybir
    import concourse.tile as tile
    from concourse import bacc

    dt = mybir.dt
    assert c_routed % 128 == 0 and t_total % CHUNK == 0
    KT_H = H // 128  # 8 k-tiles along H
    KT_F = F // 128  # 32 f-tiles (routed down-proj)
    NF_S = FS // 128  # 4 f-tiles in the shared slice

    # Bacc (not raw Bass): its compile pass splits sync waits down to the
    # TRN2 limit of 1 wait per instruction (walrus rejects multi-wait IR).
    nc = bacc.Bacc(None, target_bir_lowering=False)
    xT_r = nc.dram_tensor("xT_r", [H, c_routed], dt.float8e4, kind="ExternalInput")
    xT_s = nc.dram_tensor("xT_s", [H, t_total], dt.bfloat16, kind="ExternalInput")
    gates = nc.dram_tensor(
        "gates", [128, c_routed // 128], dt.float32, kind="ExternalInput"
    )
    w_up = nc.dram_tensor("w_up", [H, F], dt.float8e4, kind="ExternalInput")
    w_down = nc.dram_tensor("w_down", [F, H], dt.float8e4, kind="ExternalInput")
    su_s = nc.dram_tensor("su_s", [H, FS], dt.bfloat16, kind="ExternalInput")
    sd_s = nc.dram_tensor("sd_s", [FS, H], dt.bfloat16, kind="ExternalInput")
    out_r = nc.dram_tensor("out_r", [c_routed, H], dt.bfloat16, kind="ExternalOutput")
    out_s = nc.dram_tensor("out_s", [t_total, H], dt.bfloat16, kind="ExternalOutput")

    xTr_t = xT_r.rearrange("(kt p) c -> p kt c", p=128)
    xTs_t = xT_s.rearrange("(kt p) c -> p kt c", p=128)
    outr_t = out_r.rearrange("(n p) h -> p n h", p=128)
    outs_t = out_s.rearrange("(n p) h -> p n h", p=128)

    with tile.TileContext(nc) as tc, ExitStack() as ctx:
        swpool = ctx.enter_context(tc.tile_pool(name="sweights", bufs=1))
        wpool = ctx.enter_context(tc.tile_pool(name="weights", bufs=1))
        xpool = ctx.enter_context(tc.tile_pool(name="x", bufs=3))
        hpool = ctx.enter_context(tc.tile_pool(name="h", bufs=2))
        cpool = ctx.enter_context(tc.tile_pool(name="const", bufs=1))
        opool = ctx.enter_context(tc.tile_pool(name="out", bufs=6))
        # 3 pup + 4 pdown = 7 of 8 PSUM banks; using all 8 banks crashes the
        # device (NRT_EXEC_UNIT_UNRECOVERABLE) — do not fill PSUM.
        pup = ctx.enter_context(tc.tile_pool(name="pup", bufs=3, space="PSUM"))
        pdown = ctx.enter_context(tc.tile_pool(name="pdown", bufs=2, space="PSUM"))
        pools = (xpool, hpool, opool, pup, pdown)

        # shared-slice weights on the ACT DMA ring so they transfer in
        # parallel with the first x chunk (which goes on the SP ring); su in
        # two kt-halves so the f=0 matmul chain can start on the first half.
        # sd is only needed by down(chunk0), ~25us in — it queues behind su.
        su_all = swpool.tile([128, KT_H, FS], dt.bfloat16, tag="su")
        su_r = su_s.rearrange("(kt p) f -> p kt f", p=128)
        nc.scalar.dma_start(su_all[:, : KT_H // 2, :], su_r[:, : KT_H // 2, :])
        nc.scalar.dma_start(su_all[:, KT_H // 2 :, :], su_r[:, KT_H // 2 :, :])
        sd_all = swpool.tile([128, NF_S, H], dt.bfloat16, tag="sd")
        nc.scalar.dma_start(sd_all[:], sd_s.rearrange("(ft p) h -> p ft h", p=128)[:])

        # routed fp8 weights (8MB): tiles allocated now, DMAs deferred — they
        # are emitted paced across the shared phase (on the SWDGE queue) so
        # they don't steal HBM bandwidth from the shared phase's startup.
        w_dma_fns = []
        wu_all = wpool.tile([128, KT_H, F], dt.float8e4, tag="wu")
        wu_t = w_up.rearrange("(kt p) f -> p kt f", p=128)
        for kt in range(KT_H):
            w_dma_fns.append(
                lambda kt=kt: nc.gpsimd.dma_start(wu_all[:, kt, :], wu_t[:, kt, :])
            )
        wd_all = wpool.tile([128, KT_F, H], dt.float8e4, tag="wd")
        wd_t = w_down.rearrange("(ft p) h -> p ft h", p=128)
        for ft in range(0, KT_F, 4):
            w_dma_fns.append(
                lambda ft=ft: nc.gpsimd.dma_start(
                    wd_all[:, ft : ft + 4, :], wd_t[:, ft : ft + 4, :]
                )
            )

        # phase S: partial shared FFN over all tokens, F-slice FS, bf16
        _ffn_phase(nc, tile, dt, wu=su_all, wd=sd_all, x_r=xTs_t, out_r=outs_t,
                   c_lo=0, c_hi=t_total, n_f=NF_S, kstep=1, pools=pools,
                   htag="sh", hdt=dt.bfloat16, xdt=dt.bfloat16,
                   paced_dmas=w_dma_fns)

        g_sb = cpool.tile([128, c_routed // 128], dt.float32)
        nc.sync.dma_start(g_sb[:], gates[:])
        # phase R: routed expert over gathered tokens, fp8 DoubleRow, gated
        _ffn_phase(nc, tile, dt, wu=wu_all, wd=wd_all, x_r=xTr_t, out_r=outr_t,
                   c_lo=0, c_hi=c_routed, n_f=KT_F, kstep=2, pools=pools,
                   htag="rh", hdt=dt.float8e4, xdt=dt.float8e4,
                   gelu_scale=1.0 / (SX * SW), g_sb=g_sb)

    nc.finalize()
    return nc


def _get_nc(c_routed, t_total):
    key = (c_routed, t_total)
    if key not in _nc_cache:
        _nc_cache[key] = _build_nc(c_routed, t_total)
    return _nc_cache[key]


def _route(xf, router_w):
    """Host router in f64: top-2 indices (jax tie-break: lower index first)
    and their softmax probs."""
    logits = xf.astype(np.float64) @ router_w.astype(np.float64)
    m = logits.max(-1, keepdims=True)
    p = np.exp(logits - m)
    p /= p.sum(-1, keepdims=True)
    order = np.argsort(-p, axis=-1, kind="stable")
    top_idx = order[:, :TOPK]
    top_p = np.take_along_axis(p, top_idx, -1).astype(np.float32)
    return top_idx, top_p


def _fp8(v, scale):
    return np.clip(v * scale, -240.0, 240.0).astype(FP8)


def kernel(**inputs):
    x = np.ascontiguousarray(np.asarray(inputs["x"], np.float32))
    shared_up = np.asarray(inputs["shared_up"], np.float32)[0]
    shared_down = np.asarray(inputs["shared_down"], np.float32)[0]
    routed_up = np.asarray(inputs["routed_up"], np.float32)
    routed_down = np.asarray(inputs["routed_down"], np.float32)
    router_w = np.asarray(inputs["router_w"], np.float32)

    B, S, _ = x.shape
    T = B * S
    xf = x.reshape(T, H)

    top_idx, top_p = _route(xf, router_w)

    token_lists = [np.where((top_idx == e).any(-1))[0] for e in range(E)]
    c_cap = max(128, -(-max(len(l) for l in token_lists) // 128) * 128)

    # position of (token, slot) inside its expert's gathered buffer
    pos = np.zeros((T, TOPK), np.int64)
    gates_per_e = np.zeros((E, c_cap), np.float32)
    for e in range(E):
        lst = token_lists[e]
        for k in range(TOPK):
            sel = np.where(top_idx[:, k] == e)[0]
            p_in = np.searchsorted(lst, sel)
            pos[sel, k] = p_in
            gates_per_e[e, p_in] = top_p[sel, k]
    gates_per_e *= 1.0 / SW  # fold the fp8 down-weight scale into the gates

    xf_bf = xf.astype(BF16)
    xTs = np.ascontiguousarray(xf_bf.T)  # [H, T], shared phase input
    xf_q = _fp8(xf, SX)  # [T, H] fp8, routed phase input
    su_bf = shared_up.astype(BF16)
    sd_bf = shared_down.astype(BF16)

    in_maps = []
    for e in range(E):
        lst = token_lists[e]
        xe = np.zeros((c_cap, H), FP8)
        xe[: len(lst)] = xf_q[lst]
        in_maps.append(
            {
                "xT_r": np.ascontiguousarray(xe.T),
                "xT_s": xTs,
                "gates": np.ascontiguousarray(
                    gates_per_e[e].reshape(c_cap // 128, 128).T
                ),
                "w_up": _fp8(routed_up[e], SW),
                "w_down": _fp8(routed_down[e], SW),
                "su_s": np.ascontiguousarray(su_bf[:, e * FS : (e + 1) * FS]),
                "sd_s": np.ascontiguousarray(sd_bf[e * FS : (e + 1) * FS, :]),
            }
        )

    from concourse.bass_utils import run_bass_kernel_spmd

    nc = _get_nc(c_cap, T)
    res = run_bass_kernel_spmd(nc, in_maps, list(range(N_CORES)), trace=TRACE)
    global LAST_RESULT
    LAST_RESULT = res

    y = xf.copy()
    for e in range(E):
        y += res.results[e]["out_s"].astype(np.float32)
    y_routed = np.stack(
        [res.results[e]["out_r"].astype(np.float32) for e in range(E)]
    )  # gated rows
    for k in range(TOPK):
        y += y_routed[top_idx[:, k], pos[:, k]]
    return y.reshape(B, S, H)
